# revision 10
# baseline (speedup 1.0000x reference)
"""Conditional_Embedding_Contrastive_loss Trainium2 kernel.

Full-input contract: kernel(**inputs) takes the complete tensors, shards
rows across 8 NeuronCores (data-parallel), runs one SPMD Bass/Tile kernel,
and reduces the per-row log-ratios to the scalar loss on the host.

Math (reference, augmentation=None branch):
    sim   = cosine_sim(X, X)                      # [N,N]
    IZ    = exp(offdiag(sim)/T)                   # [N,N-1]
    Mneg  = offdiag(cls_mask[labels])             # [N,N-1]
    p     = exp(cos(x_i, a_i)/T)                  # [N]
    num_i = sum_j IZ*Mneg + p_i
    den_i = p_i + sum_j IZ
    loss  = -mean(log(num_i/den_i))

Since cos(x,x) == 1 exactly, the diagonal removal is analytic:
    sum_offdiag exp(sim/T)        = S_all_i - exp(1/T)
    sum_offdiag exp(sim/T)*m      = S_msk_i - exp(1/T)*m_ii
so each core computes full row sums of its [R, N] slice of exp(sim/T)
(and the masked variant) plus p_i, then:
    logq_i = ln(S_msk_i - E0*m_ii + p_i) - ln(S_all_i - E0 + p_i)
Host: loss = -mean(logq).

Device pipeline per core (R = N/8 = 512 rows):
  - G tile [128,512] = lhsT.T @ rhs over 8 k-chunks (bf16 PE matmul);
    lhsT = own-column slice of X^T, rhs = full X^T (both SBUF resident).
  - norms: squares (DVE) + ones-matmul partition-reduce (PE) -> n2 [1,N];
    r = exp(-0.5*ln(n2)) on ACT (stays in the ln/exp table set);
    r broadcast to [128,N] via stride-0 DMA through a DRAM scratch.
  - per tile: h = (G * r_i) * r_j (one DVE scalar_tensor_tensor),
    e = exp(h/T) on ACT with accum_out = unmasked row-sum (free),
    masked row-sum via one DVE tensor_tensor_reduce against the
    host-gathered mask slice cls_mask[labels[rows]].
  - p_i from row-major own slices: dots/norms via DVE reduce, exp on ACT.
"""

import sys

for _p in ("/opt/trn_rl_repo",):
    if _p not in sys.path:
        sys.path.insert(0, _p)

import numpy as np
import ml_dtypes

P = 128  # SBUF partitions
JW = 512  # j-tile width (one PSUM bank of fp32)

_CACHE = {}


def build_kernel(N, D, R, inv_T, mm_bf16=True, n_cores=8):
    """Build the SPMD Bass program for one core owning R rows of N total."""
    import concourse.bass as bass
    import concourse.mybir as mybir
    import concourse.tile as tile
    from concourse import bacc

    f32 = mybir.dt.float32
    bf16 = mybir.dt.bfloat16
    MMDT = bf16 if mm_bf16 else f32
    Exp = mybir.ActivationFunctionType.Exp
    Ln = mybir.ActivationFunctionType.Ln
    mult = mybir.AluOpType.mult
    add = mybir.AluOpType.add
    X = mybir.AxisListType.X

    E0 = float(np.exp(inv_T))  # exp(1/T): the analytic diagonal term

    KC = D // P  # contraction chunks
    NB = R // P  # own row blocks
    JC = N // JW  # j tiles
    # norm phase processes j in halves so its PSUM use peaks at JC//2 banks
    JH = 2 if JC >= 2 else 1
    JHW = N // JH
    QN = JHW // JW  # psum banks per half

    nc = bacc.Bacc(
        "TRN2", target_bir_lowering=False, debug=False, num_devices=n_cores)
    xt_d = nc.declare_dram_parameter("xt", [D, N], MMDT, isOutput=False)
    xst_d = nc.declare_dram_parameter("xst", [D, R], MMDT, isOutput=False)
    xs_d = nc.declare_dram_parameter("xs", [R, D], f32, isOutput=False)
    anc_d = nc.declare_dram_parameter("anc", [R, D], f32, isOutput=False)
    mt_d = nc.declare_dram_parameter("mt", [R, N], bf16, isOutput=False)
    md_d = nc.declare_dram_parameter("mdiag", [NB, P, 1], f32, isOutput=False)
    out_d = nc.declare_dram_parameter("logq", [NB, P, 1], f32, isOutput=True)

    with tile.TileContext(nc) as tc:
        with (
            tc.tile_pool(name="big", bufs=1) as big,
            tc.tile_pool(name="sq", bufs=2) as sqp,
            tc.tile_pool(name="mask", bufs=4) as maskp,
            tc.tile_pool(name="work", bufs=3) as workp,
            tc.tile_pool(name="stats", bufs=1) as statsp,
            tc.tile_pool(name="tiny", bufs=2) as tinyp,
            tc.tile_pool(name="rdr", bufs=1, space="DRAM") as dramp,
            tc.tile_pool(name="npsum", bufs=1, space="PSUM") as npsum,
            tc.tile_pool(name="mpsum", bufs=3, space="PSUM") as mpsum,
        ):
            xt_sb = big.tile([P, KC, N], MMDT)
            xst_sb = big.tile([P, KC, R], MMDT)
            xs_sb = big.tile([P, NB, D], f32)
            anc_sb = big.tile([P, NB, D], f32)
            rbc = big.tile([P, N], f32)
            md_sb = statsp.tile([P, NB], f32)
            ones_w = statsp.tile([P, 1], MMDT)
            accA = statsp.tile([P, NB, JC], f32)
            accM = statsp.tile([P, NB, JC], f32)
            rq = statsp.tile([P, NB], f32)  # r_i = 1/||x_i||
            pvec = statsp.tile([P, NB], f32)  # p_i
            logq = statsp.tile([P, NB], f32)
            rdram = dramp.tile([1, N], f32)

            # ---- input DMAs (xt on the HW queue; the rest on gpsimd) ----
            for c in range(KC):
                nc.sync.dma_start(xt_sb[:, c, :], xt_d[c * P : (c + 1) * P, :])
            for c in range(KC):
                nc.gpsimd.dma_start(xst_sb[:, c, :], xst_d[c * P : (c + 1) * P, :])
            for b in range(NB):
                nc.gpsimd.dma_start(xs_sb[:, b, :], xs_d[b * P : (b + 1) * P, :])
                nc.gpsimd.dma_start(anc_sb[:, b, :], anc_d[b * P : (b + 1) * P, :])
                nc.gpsimd.dma_start(md_sb[:, b : b + 1], md_d[b])
            nc.vector.memset(ones_w[:], 1.0)

            # Pre-place the combined ln+exp activation table so the compiler
            # doesn't flip-flop between the exp-only and ln-only sets
            # (each switch costs ~2.7us on the scalar engine).
            ACT_SET_LN_EXP = 6  # natural_log_exp_and_others (gen3 act_info)
            nc.scalar.add_instruction(mybir.InstLoadActFuncSet(
                name=nc.get_next_instruction_name(),
                act_func_set_id=ACT_SET_LN_EXP, ins=[], outs=[]))

            # ---- p path: p_i = exp(dot_i / (n_i * na_i * T)); also r_i ----
            for b in range(NB):
                xb = xs_sb[:, b, :]
                ab = anc_sb[:, b, :]
                n2x = tinyp.tile([P, 1], f32, tag="n2x")
                n2a = tinyp.tile([P, 1], f32, tag="n2a")
                dotv = tinyp.tile([P, 1], f32, tag="dotv")
                Sq = mybir.ActivationFunctionType.Square
                j1 = workp.tile([P, D], f32, tag="pjunk")
                nc.scalar.activation(j1, xb, Sq, accum_out=n2x)
                j2 = workp.tile([P, D], f32, tag="pjunk")
                nc.scalar.activation(j2, ab, Sq, accum_out=n2a)
                j3 = workp.tile([P, D], f32, tag="pjunk")
                nc.vector.scalar_tensor_tensor(
                    out=j3, in0=xb, scalar=1.0, in1=ab, op0=mult, op1=mult,
                    accum_out=dotv)
                l1 = tinyp.tile([P, 1], f32, tag="l1")
                l2 = tinyp.tile([P, 1], f32, tag="l2")
                nc.scalar.activation(l1, n2x, Ln)
                nc.scalar.activation(l2, n2a, Ln)
                # r_i = exp(-0.5*ln(n2x))
                nc.scalar.activation(rq[:, b : b + 1], l1, Exp, scale=-0.5)
                ls = tinyp.tile([P, 1], f32, tag="ls")
                nc.vector.tensor_add(ls, l1, l2)
                qv = tinyp.tile([P, 1], f32, tag="qv")
                nc.scalar.activation(qv, ls, Exp, scale=-0.5)  # 1/(n_i*na_i)
                q2 = tinyp.tile([P, 1], f32, tag="q2")
                nc.vector.tensor_scalar_mul(q2, qv, float(inv_T))
                nc.scalar.activation(pvec[:, b : b + 1], dotv, Exp, scale=q2)

            # ---- norms of all N columns -> r broadcast tile ----
            for jh in range(JH):
                nts = [npsum.tile([1, JW], f32, tag=f"n2q{q}", name=f"n2q{q}")
                       for q in range(QN)]
                for c in range(KC):
                    sqt = sqp.tile([P, JHW], MMDT, tag="sqt")
                    xsl = xt_sb[:, c, jh * JHW : (jh + 1) * JHW]
                    nc.vector.tensor_mul(sqt, xsl, xsl)
                    for q in range(QN):
                        nc.tensor.matmul(
                            nts[q][:], ones_w[:], sqt[:, q * JW : (q + 1) * JW],
                            start=(c == 0), stop=(c == KC - 1))
                for q in range(QN):
                    jc = jh * QN + q
                    lnr = tinyp.tile([1, JW], f32, tag="lnr")
                    nc.scalar.activation(lnr, nts[q][:], Ln)
                    rr = tinyp.tile([1, JW], f32, tag="rr")
                    nc.scalar.activation(rr, lnr, Exp, scale=-0.5)
                    nc.gpsimd.dma_start(rdram[0:1, jc * JW : (jc + 1) * JW], rr)
                    rsl = rdram[0:1, jc * JW : (jc + 1) * JW]
                    bc = bass.AP(tensor=rsl.tensor, offset=rsl.offset,
                                 ap=[[0, P], [1, JW]])
                    nc.gpsimd.dma_start(rbc[:, jc * JW : (jc + 1) * JW], bc)

            # ---- main: G tiles -> exp -> masked/unmasked row sums ----
            for b in range(NB):
                for jc in range(JC):
                    ps = mpsum.tile([P, JW], f32, tag="ps")
                    for c in range(KC):
                        nc.tensor.matmul(
                            ps[:],
                            xst_sb[:, c, b * P : (b + 1) * P],
                            xt_sb[:, c, jc * JW : (jc + 1) * JW],
                            start=(c == 0), stop=(c == KC - 1))
                    mtt = maskp.tile([P, JW], bf16, tag="mtt")
                    nc.sync.dma_start(
                        mtt, mt_d[b * P : (b + 1) * P, jc * JW : (jc + 1) * JW])
                    h = workp.tile([P, JW], f32, tag="h")
                    nc.vector.scalar_tensor_tensor(
                        out=h, in0=ps[:], scalar=rq[:, b : b + 1],
                        in1=rbc[:, jc * JW : (jc + 1) * JW], op0=mult, op1=mult)
                    e = workp.tile([P, JW], f32, tag="e")
                    nc.scalar.activation(
                        e, h, Exp, scale=float(inv_T),
                        accum_out=accA[:, b, jc : jc + 1])
                    j4 = workp.tile([P, JW], f32, tag="mjunk")
                    nc.vector.scalar_tensor_tensor(
                        out=j4, in0=e, scalar=1.0, in1=mtt, op0=mult, op1=mult,
                        accum_out=accM[:, b, jc : jc + 1])
                # tail: assemble logq for block b
                sA = tinyp.tile([P, 1], f32, tag="sA")
                sM = tinyp.tile([P, 1], f32, tag="sM")
                nc.vector.reduce_sum(sA, accA[:, b, :], axis=X)
                nc.vector.reduce_sum(sM, accM[:, b, :], axis=X)
                num = tinyp.tile([P, 1], f32, tag="num")
                # num = sM - E0*mdiag  (then + p)
                nc.vector.scalar_tensor_tensor(
                    out=num, in0=md_sb[:, b : b + 1], scalar=-E0, in1=sM,
                    op0=mult, op1=add)
                num2 = tinyp.tile([P, 1], f32, tag="num2")
                nc.vector.tensor_add(num2, num, pvec[:, b : b + 1])
                den = tinyp.tile([P, 1], f32, tag="den")
                nc.vector.tensor_add(den, sA, pvec[:, b : b + 1])
                den2 = tinyp.tile([P, 1], f32, tag="den2")
                nc.vector.tensor_scalar_add(den2, den, -E0)
                lnn = tinyp.tile([P, 1], f32, tag="lnn")
                lnd = tinyp.tile([P, 1], f32, tag="lnd")
                nc.scalar.activation(lnn, num2, Ln)
                nc.scalar.activation(lnd, den2, Ln)
                nc.vector.tensor_sub(logq[:, b : b + 1], lnn, lnd)
                nc.sync.dma_start(out_d[b], logq[:, b : b + 1])

    nc.compile()
    return nc


def _prepare_inputs(inst_embed, anchor, cls_mask, labels, n_cores):
    """Host-side sharding/marshalling: slices, transpose, mask gather, casts."""
    N, D = inst_embed.shape
    R = N // n_cores
    bf = ml_dtypes.bfloat16
    Xf = np.ascontiguousarray(inst_embed, dtype=np.float32)
    Af = np.ascontiguousarray(anchor, dtype=np.float32)
    XT = np.ascontiguousarray(Xf.T).astype(bf)
    lab = np.asarray(labels).astype(np.int64)
    in_maps = []
    for k in range(n_cores):
        r0 = k * R
        rows = slice(r0, r0 + R)
        mrows = cls_mask[lab[rows]]  # [R, N] int
        mdiag = mrows[np.arange(R), r0 + np.arange(R)].astype(np.float32)
        in_maps.append({
            "xt": XT,
            "xst": np.ascontiguousarray(XT[:, rows]),
            "xs": np.ascontiguousarray(Xf[rows]),
            "anc": np.ascontiguousarray(Af[rows]),
            "mt": np.ascontiguousarray(mrows.astype(bf)),
            "mdiag": np.ascontiguousarray(
                mdiag.reshape(R // P, P, 1)),
        })
    return in_maps


def run(inst_embed, anchor, cls_mask, labels, temperature,
        n_cores=8, trace=False, mm_bf16=True):
    """Build (cached), run on hardware, and reduce. Returns (loss, results)."""
    from concourse.bass_utils import run_bass_kernel_spmd

    N, D = inst_embed.shape
    R = N // n_cores
    inv_T = float(1.0 / np.float32(temperature))
    key = (N, D, R, inv_T, mm_bf16)
    if key not in _CACHE:
        _CACHE[key] = build_kernel(
            N, D, R, inv_T, mm_bf16=mm_bf16, n_cores=n_cores)
    nc = _CACHE[key]

    in_maps = _prepare_inputs(inst_embed, anchor, cls_mask, labels, n_cores)
    from concourse.bass_interp import get_hw_module
    hw_m = get_hw_module(nc.m)
    old_m = nc.m
    nc.m = hw_m
    try:
        res = run_bass_kernel_spmd(
            nc, in_maps, list(range(n_cores)), trace=trace)
    finally:
        nc.m = old_m
    vals = np.concatenate(
        [np.asarray(r["logq"], dtype=np.float32).reshape(-1) for r in res.results])
    loss = -np.mean(vals.astype(np.float64))
    return np.array(loss, dtype=np.float32), res


def kernel(inst_embed, anchor, cls_mask, labels, temperature):
    loss, _ = run(inst_embed, anchor, cls_mask, labels, temperature)
    return loss
